# revision 8
# baseline (speedup 1.0000x reference)
"""Trainium2 Bass kernel for the DTI R-GCN (bdd) model, 8 NeuronCores.

Strategy (SPMD, one program, per-core data):
  - dst-shard the graph: core c owns nodes [c*2500, (c+1)*2500); host routes
    each edge to its dst owner and buckets it into (dst-tile, relation) cells,
    tiles of 256 dst nodes (TILE2).
  - edge chunks of 128 are gathered in batches of G=8 chunks per
    indirect-DMA call (amortizes the ~1us SWDGE fixed cost) from a bf16 copy
    of the node features; a scatter matrix S[e, d] = norm_e * (iota ==
    dstloc_e) is built per chunk (fp32), and aggT[feat, dst] accumulates in
    PSUM via matmul(lhsT=xg_half[bf16], rhs=S[f32r]) at 1 cycle/row.
  - per dst-tile, msgT[feat_out, dst 256] accumulates in PSUM: the self-loop
    term matmul(lhsT=loopw[f32r], rhs=xT[f32r]) plus one matmul per relation
    matmul(lhsT=Wblk_r[f32r], rhs=aggT_sb[f32r]) -- all free-dim 256 so f32r
    runs at full rate with full fp32 data.  msgT is PE-transposed back,
    bias-added, and stored (fp32 own rows for the next layer's self-loop +
    bf16 rows for the next layer's gathers).
  - layer outputs are AllGather'd piecewise (layer1 in bf16 for the gathers,
    layer2 in fp32 for the MLP head).
  - MLP head is data-parallel over pairs: all 512 per-core pairs in one pass
    with free-dim-512 f32r matmuls.

The program is built fresh per invocation from the actual inputs (host does
all index preprocessing; trip counts are data-dependent but identical across
cores by padding cells to the max chunk count over cores).
"""
import sys

sys.path.insert(0, "/opt/trn_rl_repo")
import numpy as np
import ml_dtypes

P = 128
TILE2 = 2 * P  # dst nodes per aggregation cell


def _round_f32r(x):
    """Round fp32 -> FP32R (1s/8e/11m, low 12 mantissa bits zero, RNE) so
    DRAM uploads declared float32r hold already-rounded values."""
    u = np.ascontiguousarray(x, np.float32).view(np.uint32)
    u = (u + 0x7FF + ((u >> 12) & 1)) & np.uint32(0xFFFFF000)
    return u.view(np.float32)

G = 8  # chunks gathered per indirect-DMA call
# aggregation matmul dtype: bf16 halves gather DMA + S-build cost but rounds
# the gathered features and edge norms; f32(r) is exact (walrus forbids
# mixing, so gathers/S/matmul all follow this flag together)
AGG_BF16 = True
NCORES = 8
OOB = np.int32(2**28)


def _preprocess(inputs, ncores=NCORES):
    node_ids = np.asarray(inputs["node_ids"])
    src = np.asarray(inputs["src"])
    dst = np.asarray(inputs["dst"])
    etype = np.asarray(inputs["etype"])
    norm = np.asarray(inputs["norm"]).reshape(-1)
    emb = np.asarray(inputs["emb"], dtype=np.float32)
    drugs = np.asarray(inputs["drugs_index"])
    targets = np.asarray(inputs["targets_index"])

    N = node_ids.shape[0]
    H = emb.shape[1]
    R = int(inputs["w1"].shape[0])
    PAIRS = drugs.shape[0]
    assert N % ncores == 0 and PAIRS % ncores == 0
    NOWN = N // ncores
    TILES = -(-NOWN // P)
    NCELL = (-(-NOWN // TILE2)) * R
    PPC = PAIRS // ncores
    assert PPC % P == 0
    Q = PPC // P

    T2 = -(-NOWN // TILE2)
    owner = dst // NOWN
    d_local = dst - owner * NOWN
    t_of_e = d_local // TILE2
    dstloc_of_e = (d_local % TILE2).astype(np.float32)
    cell_of_e = t_of_e * R + etype

    counts = np.zeros((ncores, NCELL), np.int64)
    for c in range(ncores):
        counts[c] = np.bincount(cell_of_e[owner == c], minlength=NCELL)
    nch = -(-counts.max(axis=0) // P)  # chunks per cell (0 if empty everywhere)
    chunk_start = np.zeros(NCELL, np.int64)
    chunk_start[1:] = np.cumsum(nch)[:-1]
    TC = int(nch.sum())

    srcT = np.zeros((ncores, P, TC), np.int32)  # pads point at row 0 (S zeroes them)
    dstlocT = np.zeros((ncores, P, TC), np.float32)
    normT = np.zeros((ncores, P, TC), np.float32)
    for c in range(ncores):
        m = owner == c
        eidx = np.where(m)[0]
        cell = cell_of_e[eidx]
        order = np.argsort(cell, kind="stable")
        eidx = eidx[order]
        cell = cell[order]
        cstart = np.zeros(NCELL, np.int64)
        cstart[1:] = np.cumsum(counts[c])[:-1]
        rank = np.arange(len(eidx)) - cstart[cell]
        col = chunk_start[cell] + rank // P
        part = rank % P
        srcT[c, part, col] = src[eidx]
        dstlocT[c, part, col] = dstloc_of_e[eidx]
        normT[c, part, col] = norm[eidx]

    # host-side embedding lookup: pure data movement, shrinks per-core upload
    # from the full table to the active [N, H] slab
    h0 = emb[node_ids]  # [N, H]
    h0b = h0.astype(ml_dtypes.bfloat16) if AGG_BF16 else _round_f32r(h0)  # gather table
    x0own = np.zeros((ncores, TILES * P, H), np.float32)
    for c in range(ncores):
        x0own[c, :NOWN] = h0[c * NOWN : (c + 1) * NOWN]

    def _wrap16(flat):
        # dma_gather index layout: flat j -> partition j%16, column j//16,
        # replicated across all 128 partitions
        assert len(flat) % 16 == 0
        w = np.asarray(flat, np.int16).reshape(-1, 16).T.copy()
        return np.tile(w, (8, 1))

    srcW = np.stack([_wrap16(srcT[c].T.ravel()) for c in range(ncores)])
    drugsW = np.stack([_wrap16(drugs[c * PPC : (c + 1) * PPC]) for c in range(ncores)])
    targetsW = np.stack([_wrap16(targets[c * PPC : (c + 1) * PPC]) for c in range(ncores)])

    # relation block weights as lhsT [if_local, of_local] per (layer, rel, half)
    B = int(inputs["w1"].shape[1])
    si = H // B
    hb = (P // si)  # blocks per half
    wblk = np.zeros((2, R, 2, P, P), np.float32)
    for l, W in enumerate([inputs["w1"], inputs["w2"]]):
        W = np.asarray(W, np.float32)
        for r in range(R):
            for h in range(2):
                for bb in range(hb):
                    b = hb * h + bb
                    wblk[l, r, h, bb * si : (bb + 1) * si, bb * si : (bb + 1) * si] = W[r, b]
    wblk_in = _round_f32r(wblk.transpose(3, 0, 1, 2, 4).reshape(P, 2 * R * 2 * P))

    loopw = np.stack(
        [np.asarray(inputs["loop_w1"], np.float32), np.asarray(inputs["loop_w2"], np.float32)]
    )  # [2, H, H]
    loopw_in = _round_f32r(loopw.reshape(2, 2, P, H).transpose(2, 0, 1, 3).reshape(P, 2 * 2 * H))

    bias_in = np.concatenate(
        [
            np.tile(np.asarray(inputs["b1"], np.float32)[None, :], (P, 1)),
            np.tile(np.asarray(inputs["b2"], np.float32)[None, :], (P, 1)),
        ],
        axis=1,
    )  # [P, 2H]

    d2 = 2 * H
    KC = d2 // P  # fc1 contraction chunks
    MC = d2 // P  # fc1 output chunks
    fc1_in = _round_f32r(
        np.asarray(inputs["fc1_W"], np.float32)
        .reshape(KC, P, MC, P)
        .transpose(1, 0, 2, 3)
        .reshape(P, KC * MC * P)
    )
    fc1b_in = np.asarray(inputs["fc1_b"], np.float32).reshape(MC, P).T.copy()
    fc2_in = _round_f32r(np.asarray(inputs["fc2_W"], np.float32).reshape(MC, P).T)
    fc2b = float(np.asarray(inputs["fc2_b"]).reshape(-1)[0])

    iota = np.tile(np.arange(TILE2, dtype=np.float32), (P, 1))

    meta = dict(
        N=N, H=H, R=R, NOWN=NOWN, TILES=TILES, T2=T2, NCELL=NCELL, TC=TC, Q=Q,
        KC=KC, MC=MC, nch=nch, chunk_start=chunk_start, fc2b=fc2b,
    )
    shared = dict(
        h0b=h0b, iota=iota, wblk=wblk_in, loopw=loopw_in, biasbc=bias_in,
        fc1=fc1_in, fc1b=fc1b_in, fc2=fc2_in,
    )
    in_maps = []
    for c in range(ncores):
        m = dict(shared)
        m.update(
            srcW=srcW[c], dstlocT=dstlocT[c], normT=normT[c],
            x0own=x0own[c], drugsW=drugsW[c], targetsW=targetsW[c],
        )
        in_maps.append(m)
    return meta, in_maps


def _build(meta, ncores=NCORES, single=False):
    from concourse import bass, mybir, tile, bacc
    from concourse.masks import make_identity

    N, H, R = meta["N"], meta["H"], meta["R"]
    NOWN, TILES, TC, Q = meta["NOWN"], meta["TILES"], meta["TC"], meta["Q"]
    T2 = meta["T2"]
    KC, MC = meta["KC"], meta["MC"]
    nch, chunk_start = meta["nch"], meta["chunk_start"]
    f32 = mybir.dt.float32
    f32r = mybir.dt.float32r
    bf16 = mybir.dt.bfloat16
    i32 = mybir.dt.int32
    gdt = bf16 if AGG_BF16 else f32r  # gather-table / xg / S dtype

    nc = bacc.Bacc(
        "TRN2", target_bir_lowering=False, debug=False,
        num_devices=(1 if single else ncores),
        dynamic_dma_scratch_size=32768,
    )

    h0b_t = nc.dram_tensor("h0b", [N, H], gdt, kind="ExternalInput")
    i16 = mybir.dt.int16
    WTC = TC * P // 16
    srcW_t = nc.dram_tensor("srcW", [P, WTC], i16, kind="ExternalInput")
    dstlocT_t = nc.dram_tensor("dstlocT", [P, TC], f32, kind="ExternalInput")
    normT_t = nc.dram_tensor("normT", [P, TC], f32, kind="ExternalInput")
    x0own_t = nc.dram_tensor("x0own", [TILES * P, H], f32, kind="ExternalInput")
    WQ = Q * P // 16
    drugsW_t = nc.dram_tensor("drugsW", [P, WQ], i16, kind="ExternalInput")
    targetsW_t = nc.dram_tensor("targetsW", [P, WQ], i16, kind="ExternalInput")
    iota_t = nc.dram_tensor("iota", [P, TILE2], f32, kind="ExternalInput")
    wblk_t = nc.dram_tensor("wblk", [P, 2 * R * 2 * P], f32r, kind="ExternalInput")
    loopw_t = nc.dram_tensor("loopw", [P, 2 * 2 * H], f32r, kind="ExternalInput")
    biasbc_t = nc.dram_tensor("biasbc", [P, 2 * H], f32, kind="ExternalInput")
    fc1_t = nc.dram_tensor("fc1", [P, KC * MC * P], f32r, kind="ExternalInput")
    fc1b_t = nc.dram_tensor("fc1b", [P, MC], f32, kind="ExternalInput")
    fc2_t = nc.dram_tensor("fc2", [P, MC], f32r, kind="ExternalInput")
    out_t = nc.dram_tensor("out", [Q * P, 1], f32, kind="ExternalOutput")

    with tile.TileContext(nc) as tc:
        with (
            tc.tile_pool(name="const", bufs=1) as cp,
            tc.tile_pool(name="work", bufs=2) as wp,
            tc.tile_pool(name="ps", bufs=1, space="PSUM") as pp,
        ):
            # ---- DRAM internals; AllGather split into piece collectives so
            # each piece fires as soon as its tiles are stored (overlaps the
            # rest of the layer), then one strided DMA folds it into the
            # node-indexed full table.
            tpp = max(1, -(-TILES // 4))  # tiles per AG piece
            pieces = []  # (row0, nrows)
            for p0 in range(0, TILES, tpp):
                row0 = p0 * P
                nrows = min(NOWN, (p0 + tpp) * P) - row0
                if nrows > 0:
                    pieces.append((row0, nrows))
            h1_own = nc.dram_tensor("h1_own", [TILES * P, H], f32, kind="Internal").ap()
            h1b_full = nc.dram_tensor("h1b_full", [N, H], gdt, kind="Internal").ap()
            h2_full = nc.dram_tensor("h2_full", [N, H], f32, kind="Internal").ap()
            agdt = {1: gdt, 2: f32}
            agin = {}
            agout = {}
            for li in (1, 2):
                for pi, (row0, nrows) in enumerate(pieces):
                    agin[(li, pi)] = nc.dram_tensor(
                        f"h{li}_agin{pi}", [nrows, H], agdt[li], kind="Internal"
                    ).ap()
                    agout[(li, pi)] = nc.dram_tensor(
                        f"h{li}_agout{pi}", [ncores * nrows, H], agdt[li],
                        kind="Internal", addr_space="Shared",
                    ).ap()

            # ---- resident constants ----
            srcW = cp.tile([P, WTC], i16, name="srcW")
            nc.sync.dma_start(srcW[:], srcW_t.ap()[:])
            dstlocT = cp.tile([P, TC], f32, name="dstlocT")
            nc.sync.dma_start(dstlocT[:], dstlocT_t.ap()[:])
            normT = cp.tile([P, TC], f32, name="normT")
            nc.sync.dma_start(normT[:], normT_t.ap()[:])
            drugsW = cp.tile([P, WQ], i16, name="drugsW")
            nc.sync.dma_start(drugsW[:], drugsW_t.ap()[:])
            targetsW = cp.tile([P, WQ], i16, name="targetsW")
            nc.sync.dma_start(targetsW[:], targetsW_t.ap()[:])
            iota_sb = cp.tile([P, TILE2], f32, name="iota_sb")
            nc.sync.dma_start(iota_sb[:], iota_t.ap()[:])
            wblk_sb = cp.tile([P, 2 * R * 2 * P], f32r, name="wblk_sb")
            nc.sync.dma_start(wblk_sb[:], wblk_t.ap()[:])
            loopw_sb = cp.tile([P, 2 * 2 * H], f32r, name="loopw_sb")
            nc.sync.dma_start(loopw_sb[:], loopw_t.ap()[:])
            biasbc_sb = cp.tile([P, 2 * H], f32, name="biasbc_sb")
            nc.sync.dma_start(biasbc_sb[:], biasbc_t.ap()[:])
            fc1_sb = cp.tile([P, KC * MC * P], f32r, name="fc1_sb")
            nc.sync.dma_start(fc1_sb[:], fc1_t.ap()[:])
            fc1b_sb = cp.tile([P, MC], f32, name="fc1b_sb")
            nc.sync.dma_start(fc1b_sb[:], fc1b_t.ap()[:])
            fc2_sb = cp.tile([P, MC], f32r, name="fc2_sb")
            nc.sync.dma_start(fc2_sb[:], fc2_t.ap()[:])
            ident = cp.tile([P, P], f32, name="ident")
            make_identity(nc, ident[:])

            def wblk_ap(l, r, h):
                o = ((l * R + r) * 2 + h) * P
                return wblk_sb[:, o : o + P]

            def loopw_ap(l, h):
                o = (l * 2 + h) * H
                return loopw_sb[:, o : o + H]

            XGW = G * H  # wide gather tile columns
            NXGW = 3

            def emit_ag_piece(li, pi, h_full):
                row0, nrows = pieces[pi]
                if single:
                    nc.sync.dma_start(
                        h_full[row0 : row0 + nrows, :], agin[(li, pi)][:]
                    )
                    return
                nc.gpsimd.collective_compute(
                    "AllGather", mybir.AluOpType.bypass,
                    replica_groups=[list(range(ncores))],
                    ins=[agin[(li, pi)]], outs=[agout[(li, pi)]],
                )
                src_ap = agout[(li, pi)].rearrange("(c n) h -> c n h", c=ncores)
                dst_ap = h_full.rearrange("(c n) h -> c n h", c=ncores)[
                    :, row0 : row0 + nrows, :
                ]
                nc.sync.dma_start(dst_ap, src_ap)

            def layer(l, xsrc_ap, xsrc_rows, xown_ap, out_pad_ap, li, h_full_out):
                # gather group state: one wide tile per G chunk columns
                gstate = {"tile": None, "g0": -1}

                def xg_slice(col, h):
                    g0 = (col // G) * G
                    if gstate["g0"] != g0:
                        w = min(G, TC - g0)
                        xgw = wp.tile([P, XGW], gdt, name="xgw", tag="xgw", bufs=NXGW)
                        nc.gpsimd.dma_gather(
                            xgw[:, : w * H].rearrange("p (b e) -> p b e", e=H),
                            xsrc_ap,
                            srcW[:, g0 * (P // 16) : (g0 + w) * (P // 16)],
                            w * P, w * P, H,
                        )
                        gstate["tile"] = xgw
                        gstate["g0"] = g0
                    k = col - g0
                    return gstate["tile"][:, k * H + h * P : k * H + (h + 1) * P]

                for t2 in range(T2):
                    rels = [r for r in range(R) if nch[t2 * R + r] > 0]
                    subs = [st for st in (2 * t2, 2 * t2 + 1) if st < TILES]
                    # self-loop: transpose own rows -> xT[h] = [fi, dst 256]
                    xT = {}
                    xown_sbs = []
                    for si_, st in enumerate(subs):
                        xown_sb = wp.tile([P, H], f32, name="xown", tag="xown", bufs=2)
                        nc.sync.dma_start(xown_sb[:], xown_ap[st * P : (st + 1) * P, :])
                        xown_sbs.append(xown_sb)
                    for h in range(2):
                        tp_ps = pp.tile([P, TILE2], f32, name="tp", tag="tpx", bufs=2)
                        for si_, st in enumerate(subs):
                            nc.tensor.transpose(
                                tp_ps[:, si_ * P : (si_ + 1) * P],
                                xown_sbs[si_][:, h * P : (h + 1) * P], ident[:],
                            )
                        xT_sb = wp.tile([P, TILE2], f32r, name=f"xT{h}", tag=f"xT{h}", bufs=2)
                        if h == 0:
                            nc.vector.tensor_copy(xT_sb[:], tp_ps[:])
                        else:
                            nc.scalar.copy(xT_sb[:], tp_ps[:])
                        xT[h] = xT_sb
                    # msgT accumulators [feat_out half, dst 256]
                    msgT_ps = {}
                    for ho in range(2):
                        msgT_ps[ho] = pp.tile(
                            [P, TILE2], f32, name=f"m{ho}", tag=f"m{ho}", bufs=1
                        )
                        for h in range(2):
                            nc.tensor.matmul(
                                msgT_ps[ho][:],
                                lhsT=loopw_ap(l, h)[:, ho * P : (ho + 1) * P],
                                rhs=xT[h][:],
                                start=(h == 0), stop=(h == 1 and not rels),
                            )
                    for ri, r in enumerate(rels):
                        cell = t2 * R + r
                        cs = int(chunk_start[cell])
                        n = int(nch[cell])
                        aggT_ps = [
                            pp.tile([P, TILE2], f32, name=f"agg{h}", tag=f"agg{h}", bufs=2)
                            for h in range(2)
                        ]
                        for ci in range(n):
                            col = cs + ci
                            xg0 = xg_slice(col, 0)
                            xg1 = xg_slice(col, 1)
                            S = wp.tile([P, TILE2], gdt, name="S", tag="S", bufs=3)
                            seng = nc.vector if (col & 1) else nc.gpsimd
                            seng.tensor_scalar(
                                out=S[:], in0=iota_sb[:],
                                scalar1=dstlocT[:, col : col + 1],
                                scalar2=normT[:, col : col + 1],
                                op0=mybir.AluOpType.is_equal, op1=mybir.AluOpType.mult,
                            )
                            nc.tensor.matmul(
                                aggT_ps[0][:], lhsT=xg0, rhs=S[:],
                                start=(ci == 0), stop=(ci == n - 1),
                            )
                            nc.tensor.matmul(
                                aggT_ps[1][:], lhsT=xg1, rhs=S[:],
                                start=(ci == 0), stop=(ci == n - 1),
                            )
                        last_rel = ri == len(rels) - 1
                        for h in range(2):
                            aggT_sb = wp.tile(
                                [P, TILE2], f32r, name=f"aggsb{h}", tag=f"aggsb{h}", bufs=2
                            )
                            if h == 0:
                                nc.vector.tensor_copy(aggT_sb[:], aggT_ps[h][:])
                            else:
                                nc.scalar.copy(aggT_sb[:], aggT_ps[h][:])
                            nc.tensor.matmul(
                                msgT_ps[h][:],
                                lhsT=wblk_ap(l, r, h),
                                rhs=aggT_sb[:],
                                start=False, stop=last_rel,
                            )
                    # transpose msgT back, add bias, store
                    out_sbs = [
                        wp.tile([P, H], f32, name=f"outsb{si_}", tag=f"outsb{si_}", bufs=2)
                        for si_ in range(len(subs))
                    ]
                    for ho in range(2):
                        msgT_sb = wp.tile(
                            [P, TILE2], f32, name=f"msgsb{ho}", tag=f"msgsb{ho}", bufs=2
                        )
                        if ho == 0:
                            nc.vector.tensor_copy(msgT_sb[:], msgT_ps[ho][:])
                        else:
                            nc.scalar.copy(msgT_sb[:], msgT_ps[ho][:])
                        for si_, st in enumerate(subs):
                            tp2_ps = pp.tile([P, P], f32, name="tp2", tag="tpx", bufs=2)
                            nc.tensor.transpose(
                                tp2_ps[:], msgT_sb[:, si_ * P : (si_ + 1) * P], ident[:]
                            )
                            nc.vector.tensor_tensor(
                                out=out_sbs[si_][:, ho * P : (ho + 1) * P],
                                in0=tp2_ps[:],
                                in1=biasbc_sb[:, l * H + ho * P : l * H + (ho + 1) * P],
                                op=mybir.AluOpType.add,
                            )
                    for si_, st in enumerate(subs):
                        out_sb = out_sbs[si_]
                        if out_pad_ap is not None:
                            nc.sync.dma_start(
                                out_pad_ap[st * P : (st + 1) * P, :], out_sb[:]
                            )
                        rows = min(P, NOWN - st * P)
                        pi = st // tpp
                        off = (st - pi * tpp) * P
                        if agdt[li] != f32:
                            out_b = wp.tile([P, H], bf16, name="outb", tag="outb", bufs=2)
                            nc.scalar.copy(out_b[:], out_sb[:])
                            nc.sync.dma_start(
                                agin[(li, pi)][off : off + rows, :], out_b[:rows, :]
                            )
                        else:
                            nc.sync.dma_start(
                                agin[(li, pi)][off : off + rows, :], out_sb[:rows, :]
                            )
                        # fire this piece's AllGather as soon as its tiles are
                        # stored so the collective overlaps the rest of the layer
                        if st == TILES - 1 or (st + 1) % tpp == 0:
                            emit_ag_piece(li, pi, h_full_out)

            layer(0, h0b_t.ap()[:], N, x0own_t.ap(), h1_own, 1, h1b_full)
            layer(1, h1b_full[:], N, h1_own, None, 2, h2_full)

            # ---- MLP head: all Q*P pairs of this core in one pass ----
            # gather drug/target rows (fp32) for all Q row-blocks at once
            xdr = wp.tile([P, Q * H], f32, name="xdr", tag="xdr", bufs=1)
            nc.gpsimd.dma_gather(
                xdr[:].rearrange("p (b e) -> p b e", e=H), h2_full[:],
                drugsW[:], Q * P, Q * P, H,
            )
            xtg = wp.tile([P, Q * H], f32, name="xtg", tag="xtg", bufs=1)
            nc.gpsimd.dma_gather(
                xtg[:].rearrange("p (b e) -> p b e", e=H), h2_full[:],
                targetsW[:], Q * P, Q * P, H,
            )
            NP = Q * P  # pairs per core
            # xcatT[k] = [feat chunk k of concat(drug,target), all pairs]
            xcatT = []
            for k in range(KC):
                src_sb = xdr if k < KC // 2 else xtg
                kk = k % (KC // 2)
                xcT = wp.tile([P, NP], f32r, name=f"xcT{k}", tag=f"xcT{k}", bufs=1)
                for q in range(Q):
                    tp2_ps = pp.tile([P, P], f32, name="tp2h", tag="tpx", bufs=2)
                    nc.tensor.transpose(
                        tp2_ps[:], src_sb[:, q * H + kk * P : q * H + (kk + 1) * P],
                        ident[:],
                    )
                    if q % 2 == 0:
                        nc.vector.tensor_copy(xcT[:, q * P : (q + 1) * P], tp2_ps[:])
                    else:
                        nc.scalar.copy(xcT[:, q * P : (q + 1) * P], tp2_ps[:])
                xcatT.append(xcT)
            z_ps = pp.tile([1, NP], f32, name="z", tag="m1", bufs=1)
            for m in range(MC):
                yT_ps = pp.tile([P, NP], f32, name="yT", tag="m0", bufs=1)
                for k in range(KC):
                    nc.tensor.matmul(
                        yT_ps[:],
                        lhsT=fc1_sb[:, (k * MC + m) * P : (k * MC + m + 1) * P],
                        rhs=xcatT[k][:],
                        start=(k == 0), stop=(k == KC - 1),
                    )
                yTr = wp.tile([P, NP], f32r, name="yTr", tag="yTr", bufs=2)
                nc.scalar.activation(
                    yTr[:], yT_ps[:], mybir.ActivationFunctionType.Relu,
                    bias=fc1b_sb[:, m : m + 1], scale=1.0,
                )
                nc.tensor.matmul(
                    z_ps[:], lhsT=fc2_sb[:, m : m + 1],
                    rhs=yTr[:],
                    start=(m == 0), stop=(m == MC - 1),
                )
            zs = wp.tile([1, NP], f32, name="zs", tag="zs", bufs=1)
            nc.scalar.activation(
                zs[:], z_ps[:], mybir.ActivationFunctionType.Sigmoid,
                bias=meta["fc2b"], scale=1.0,
            )
            nc.sync.dma_start(out_t.ap()[:, :], zs[:])
    return nc


_NC_CACHE = []


def kernel(**inputs):
    from concourse import bass_utils

    meta, in_maps = _preprocess(inputs)
    key = (meta["N"], meta["H"], meta["R"], meta["TC"], meta["Q"],
           tuple(int(x) for x in meta["nch"]))
    if _NC_CACHE and _NC_CACHE[0][0] == key:
        nc = _NC_CACHE[0][1]
    else:
        nc = _build(meta)
        nc.compile()
        _NC_CACHE[:] = [(key, nc)]
    res = bass_utils.run_bass_kernel_spmd(nc, in_maps, core_ids=list(range(NCORES)))
    out = np.concatenate([res.results[c]["out"] for c in range(NCORES)], axis=0)
    return out.astype(np.float32)


# revision 20
# speedup vs baseline: 1.3326x; 1.3326x over previous
"""Trainium2 Bass kernel for the DTI R-GCN (bdd) model, 8 NeuronCores.

Strategy (SPMD, one program, per-core data):
  - dst-shard the graph: core c owns nodes [c*2500, (c+1)*2500); host routes
    each edge to its dst owner and buckets it into (dst-tile, relation) cells,
    tiles of 256 dst nodes (TILE2).
  - edge chunks of 128 are gathered in batches of G=8 chunks per
    indirect-DMA call (amortizes the ~1us SWDGE fixed cost) from a bf16 copy
    of the node features; a scatter matrix S[e, d] = norm_e * (iota ==
    dstloc_e) is built per chunk (fp32), and aggT[feat, dst] accumulates in
    PSUM via matmul(lhsT=xg_half[bf16], rhs=S[f32r]) at 1 cycle/row.
  - per dst-tile, msgT[feat_out, dst 256] accumulates in PSUM: the self-loop
    term matmul(lhsT=loopw[f32r], rhs=xT[f32r]) plus one matmul per relation
    matmul(lhsT=Wblk_r[f32r], rhs=aggT_sb[f32r]) -- all free-dim 256 so f32r
    runs at full rate with full fp32 data.  msgT is PE-transposed back,
    bias-added, and stored (fp32 own rows for the next layer's self-loop +
    bf16 rows for the next layer's gathers).
  - layer outputs are AllGather'd piecewise (layer1 in bf16 for the gathers,
    layer2 in fp32 for the MLP head).
  - MLP head is data-parallel over pairs: all 512 per-core pairs in one pass
    with free-dim-512 f32r matmuls.

The program is built fresh per invocation from the actual inputs (host does
all index preprocessing; trip counts are data-dependent but identical across
cores by padding cells to the max chunk count over cores).
"""
import sys

sys.path.insert(0, "/opt/trn_rl_repo")
import numpy as np
import ml_dtypes

P = 128
TILE2 = 2 * P  # dst nodes per aggregation cell


def _round_f32r(x):
    """Round fp32 -> FP32R (1s/8e/11m, low 12 mantissa bits zero, RNE) so
    DRAM uploads declared float32r hold already-rounded values."""
    u = np.ascontiguousarray(x, np.float32).view(np.uint32)
    u = (u + 0x7FF + ((u >> 12) & 1)) & np.uint32(0xFFFFF000)
    return u.view(np.float32)

G = 8  # chunks gathered per indirect-DMA call
# aggregation matmul dtype: bf16 halves gather DMA + S-build cost but rounds
# the gathered features and edge norms; f32(r) is exact (walrus forbids
# mixing, so gathers/S/matmul all follow this flag together)
AGG_BF16 = True
NCORES = 8
OOB = np.int32(2**28)


def _preprocess(inputs, ncores=NCORES):
    node_ids = np.asarray(inputs["node_ids"])
    src = np.asarray(inputs["src"])
    dst = np.asarray(inputs["dst"])
    etype = np.asarray(inputs["etype"])
    norm = np.asarray(inputs["norm"]).reshape(-1)
    emb = np.asarray(inputs["emb"], dtype=np.float32)
    drugs = np.asarray(inputs["drugs_index"])
    targets = np.asarray(inputs["targets_index"])

    N = node_ids.shape[0]
    H = emb.shape[1]
    R = int(inputs["w1"].shape[0])
    PAIRS = drugs.shape[0]
    assert N % ncores == 0 and PAIRS % ncores == 0
    NOWN = N // ncores
    TILES = -(-NOWN // P)
    NCELL = (-(-NOWN // TILE2)) * R
    PPC = PAIRS // ncores
    assert PPC % P == 0
    Q = PPC // P

    T2 = -(-NOWN // TILE2)
    owner = dst // NOWN
    d_local = dst - owner * NOWN
    t_of_e = d_local // TILE2
    dstloc_of_e = (d_local % TILE2).astype(np.float32)
    cell_of_e = t_of_e * R + etype

    counts = np.zeros((ncores, NCELL), np.int64)
    for c in range(ncores):
        counts[c] = np.bincount(cell_of_e[owner == c], minlength=NCELL)
    nch = -(-counts.max(axis=0) // P)  # chunks per cell (0 if empty everywhere)
    chunk_start = np.zeros(NCELL, np.int64)
    chunk_start[1:] = np.cumsum(nch)[:-1]
    TC = int(nch.sum())

    srcT = np.zeros((ncores, P, TC), np.int32)  # pads point at row 0 (S zeroes them)
    dstlocT = np.zeros((ncores, P, TC), np.float32)
    normT = np.zeros((ncores, P, TC), np.float32)
    for c in range(ncores):
        m = owner == c
        eidx = np.where(m)[0]
        cell = cell_of_e[eidx]
        order = np.argsort(cell, kind="stable")
        eidx = eidx[order]
        cell = cell[order]
        cstart = np.zeros(NCELL, np.int64)
        cstart[1:] = np.cumsum(counts[c])[:-1]
        rank = np.arange(len(eidx)) - cstart[cell]
        col = chunk_start[cell] + rank // P
        part = rank % P
        srcT[c, part, col] = src[eidx]
        dstlocT[c, part, col] = dstloc_of_e[eidx]
        normT[c, part, col] = norm[eidx]

    # host-side embedding lookup: pure data movement, shrinks per-core upload
    # from the full table to the active [N, H] slab
    h0 = emb[node_ids]  # [N, H]
    h0b = h0.astype(ml_dtypes.bfloat16) if AGG_BF16 else _round_f32r(h0)  # gather table
    x0own = np.zeros((ncores, TILES * P, H), np.float32)
    for c in range(ncores):
        x0own[c, :NOWN] = h0[c * NOWN : (c + 1) * NOWN]

    def _wrap16(flat):
        # dma_gather index layout: flat j -> partition j%16, column j//16,
        # replicated across all 128 partitions
        assert len(flat) % 16 == 0
        w = np.asarray(flat, np.int16).reshape(-1, 16).T.copy()
        return np.tile(w, (8, 1))

    srcW = np.stack([_wrap16(srcT[c].T.ravel()) for c in range(ncores)])
    drugsW = np.stack([_wrap16(drugs[c * PPC : (c + 1) * PPC]) for c in range(ncores)])
    targetsW = np.stack([_wrap16(targets[c * PPC : (c + 1) * PPC]) for c in range(ncores)])

    # relation block weights as lhsT [if_local, of_local] per (layer, rel, half)
    B = int(inputs["w1"].shape[1])
    si = H // B
    hb = (P // si)  # blocks per half
    wblk = np.zeros((2, R, 2, P, P), np.float32)
    for l, W in enumerate([inputs["w1"], inputs["w2"]]):
        W = np.asarray(W, np.float32)
        for r in range(R):
            for h in range(2):
                for bb in range(hb):
                    b = hb * h + bb
                    wblk[l, r, h, bb * si : (bb + 1) * si, bb * si : (bb + 1) * si] = W[r, b]
    wblk_in = _round_f32r(wblk.transpose(3, 0, 1, 2, 4).reshape(P, 2 * R * 2 * P))

    loopw = np.stack(
        [np.asarray(inputs["loop_w1"], np.float32), np.asarray(inputs["loop_w2"], np.float32)]
    )  # [2, H, H]
    loopw_in = _round_f32r(loopw.reshape(2, 2, P, H).transpose(2, 0, 1, 3).reshape(P, 2 * 2 * H))

    bias_in = np.concatenate(
        [
            np.tile(np.asarray(inputs["b1"], np.float32)[None, :], (P, 1)),
            np.tile(np.asarray(inputs["b2"], np.float32)[None, :], (P, 1)),
        ],
        axis=1,
    )  # [P, 2H]

    d2 = 2 * H
    KC = d2 // P  # fc1 contraction chunks
    MC = d2 // P  # fc1 output chunks
    fc1_in = _round_f32r(
        np.asarray(inputs["fc1_W"], np.float32)
        .reshape(KC, P, MC, P)
        .transpose(1, 0, 2, 3)
        .reshape(P, KC * MC * P)
    )
    fc1b_in = np.asarray(inputs["fc1_b"], np.float32).reshape(MC, P).T.copy()
    fc2_in = _round_f32r(np.asarray(inputs["fc2_W"], np.float32).reshape(MC, P).T)
    fc2b = float(np.asarray(inputs["fc2_b"]).reshape(-1)[0])

    iota = np.tile(np.arange(TILE2, dtype=ml_dtypes.bfloat16), (P, 1))

    meta = dict(
        N=N, H=H, R=R, NOWN=NOWN, TILES=TILES, T2=T2, NCELL=NCELL, TC=TC, Q=Q,
        KC=KC, MC=MC, nch=nch, chunk_start=chunk_start, fc2b=fc2b,
    )
    shared = dict(
        h0b=h0b, iota=iota, wblk=wblk_in, loopw=loopw_in, biasbc=bias_in,
        fc1=fc1_in, fc1b=fc1b_in, fc2=fc2_in,
    )
    in_maps = []
    for c in range(ncores):
        m = dict(shared)
        m.update(
            srcW=srcW[c], dstlocT=dstlocT[c], normT=normT[c],
            x0own=x0own[c], drugsW=drugsW[c], targetsW=targetsW[c],
        )
        in_maps.append(m)
    return meta, in_maps


def _build(meta, ncores=NCORES, single=False):
    from concourse import bass, mybir, tile, bacc
    from concourse.masks import make_identity

    N, H, R = meta["N"], meta["H"], meta["R"]
    NOWN, TILES, TC, Q = meta["NOWN"], meta["TILES"], meta["TC"], meta["Q"]
    T2 = meta["T2"]
    KC, MC = meta["KC"], meta["MC"]
    nch, chunk_start = meta["nch"], meta["chunk_start"]
    f32 = mybir.dt.float32
    f32r = mybir.dt.float32r
    bf16 = mybir.dt.bfloat16
    i32 = mybir.dt.int32
    gdt = bf16 if AGG_BF16 else f32r  # gather-table / xg / S dtype

    nc = bacc.Bacc(
        "TRN2", target_bir_lowering=False, debug=False,
        num_devices=(1 if single else ncores),
        dynamic_dma_scratch_size=32768,
    )

    h0b_t = nc.dram_tensor("h0b", [N, H], gdt, kind="ExternalInput")
    i16 = mybir.dt.int16
    WTC = TC * P // 16
    srcW_t = nc.dram_tensor("srcW", [P, WTC], i16, kind="ExternalInput")
    dstlocT_t = nc.dram_tensor("dstlocT", [P, TC], f32, kind="ExternalInput")
    normT_t = nc.dram_tensor("normT", [P, TC], f32, kind="ExternalInput")
    x0own_t = nc.dram_tensor("x0own", [TILES * P, H], f32, kind="ExternalInput")
    WQ = Q * P // 16
    drugsW_t = nc.dram_tensor("drugsW", [P, WQ], i16, kind="ExternalInput")
    targetsW_t = nc.dram_tensor("targetsW", [P, WQ], i16, kind="ExternalInput")
    iota_t = nc.dram_tensor("iota", [P, TILE2], bf16, kind="ExternalInput")
    wblk_t = nc.dram_tensor("wblk", [P, 2 * R * 2 * P], f32r, kind="ExternalInput")
    loopw_t = nc.dram_tensor("loopw", [P, 2 * 2 * H], f32r, kind="ExternalInput")
    biasbc_t = nc.dram_tensor("biasbc", [P, 2 * H], f32, kind="ExternalInput")
    fc1_t = nc.dram_tensor("fc1", [P, KC * MC * P], f32r, kind="ExternalInput")
    fc1b_t = nc.dram_tensor("fc1b", [P, MC], f32, kind="ExternalInput")
    fc2_t = nc.dram_tensor("fc2", [P, MC], f32r, kind="ExternalInput")
    out_t = nc.dram_tensor("out", [Q * P, 1], f32, kind="ExternalOutput")

    with tile.TileContext(nc) as tc:
        with (
            tc.tile_pool(name="const", bufs=1) as cp,
            tc.tile_pool(name="work", bufs=2) as wp,
            tc.tile_pool(name="ps", bufs=1, space="PSUM") as pp,
        ):
            # ---- DRAM internals; AllGather split into piece collectives so
            # each piece fires as soon as its tiles are stored (overlaps the
            # rest of the layer), then one strided DMA folds it into the
            # node-indexed full table.
            tpp = max(1, -(-TILES // 4))  # tiles per AG piece
            pieces = []  # (row0, nrows)
            for p0 in range(0, TILES, tpp):
                row0 = p0 * P
                nrows = min(NOWN, (p0 + tpp) * P) - row0
                if nrows > 0:
                    pieces.append((row0, nrows))
            h1_own = nc.dram_tensor("h1_own", [TILES * P, H], f32, kind="Internal").ap()
            h1b_full = nc.dram_tensor("h1b_full", [N, H], gdt, kind="Internal").ap()
            h2_full = nc.dram_tensor("h2_full", [N, H], f32, kind="Internal").ap()
            agdt = {1: gdt, 2: f32}
            agin = {}
            agout = {}
            for li in (1, 2):
                for pi, (row0, nrows) in enumerate(pieces):
                    agin[(li, pi)] = nc.dram_tensor(
                        f"h{li}_agin{pi}", [nrows, H], agdt[li], kind="Internal"
                    ).ap()
                    agout[(li, pi)] = nc.dram_tensor(
                        f"h{li}_agout{pi}", [ncores * nrows, H], agdt[li],
                        kind="Internal", addr_space="Shared",
                    ).ap()

            # ---- resident constants ----
            srcW = cp.tile([P, WTC], i16, name="srcW")
            nc.sync.dma_start(srcW[:], srcW_t.ap()[:])
            dstlocT = cp.tile([P, TC], f32, name="dstlocT")
            nc.sync.dma_start(dstlocT[:], dstlocT_t.ap()[:])
            normT = cp.tile([P, TC], f32, name="normT")
            nc.sync.dma_start(normT[:], normT_t.ap()[:])
            drugsW = cp.tile([P, WQ], i16, name="drugsW")
            nc.sync.dma_start(drugsW[:], drugsW_t.ap()[:])
            targetsW = cp.tile([P, WQ], i16, name="targetsW")
            nc.sync.dma_start(targetsW[:], targetsW_t.ap()[:])
            iota_sb = cp.tile([P, TILE2], bf16, name="iota_sb")
            nc.sync.dma_start(iota_sb[:], iota_t.ap()[:])
            wblk_sb = cp.tile([P, 2 * R * 2 * P], f32r, name="wblk_sb")
            nc.sync.dma_start(wblk_sb[:], wblk_t.ap()[:])
            loopw_sb = cp.tile([P, 2 * 2 * H], f32r, name="loopw_sb")
            nc.sync.dma_start(loopw_sb[:], loopw_t.ap()[:])
            biasbc_sb = cp.tile([P, 2 * H], f32, name="biasbc_sb")
            nc.sync.dma_start(biasbc_sb[:], biasbc_t.ap()[:])
            fc1_sb = cp.tile([P, KC * MC * P], f32r, name="fc1_sb")
            nc.sync.dma_start(fc1_sb[:], fc1_t.ap()[:])
            fc1b_sb = cp.tile([P, MC], f32, name="fc1b_sb")
            nc.sync.dma_start(fc1b_sb[:], fc1b_t.ap()[:])
            fc2_sb = cp.tile([P, MC], f32r, name="fc2_sb")
            nc.sync.dma_start(fc2_sb[:], fc2_t.ap()[:])
            ident = cp.tile([P, P], f32, name="ident")
            make_identity(nc, ident[:])

            def wblk_ap(l, r, h):
                o = ((l * R + r) * 2 + h) * P
                return wblk_sb[:, o : o + P]

            def loopw_ap(l, h):
                o = (l * 2 + h) * H
                return loopw_sb[:, o : o + H]

            XGW = G * H  # wide gather tile columns
            NXGW = 6

            def emit_ag_piece(li, pi, h_full):
                row0, nrows = pieces[pi]
                if single:
                    nc.sync.dma_start(
                        h_full[row0 : row0 + nrows, :], agin[(li, pi)][:]
                    )
                    return
                nc.gpsimd.collective_compute(
                    "AllGather", mybir.AluOpType.bypass,
                    replica_groups=[list(range(ncores))],
                    ins=[agin[(li, pi)]], outs=[agout[(li, pi)]],
                )
                src_ap = agout[(li, pi)].rearrange("(c n) h -> c n h", c=ncores)
                dst_ap = h_full.rearrange("(c n) h -> c n h", c=ncores)[
                    :, row0 : row0 + nrows, :
                ]
                nc.sync.dma_start(dst_ap, src_ap)

            def layer(l, xsrc_ap, xsrc_rows, xown_ap, out_pad_ap, li, h_full_out):
                # gather groups of G chunk columns, prefetched one group ahead
                gtiles = {}

                def issue_gather(g0):
                    if g0 in gtiles or g0 >= TC:
                        return
                    w = min(G, TC - g0)
                    xgw = wp.tile([P, XGW], gdt, name="xgw", tag="xgw", bufs=NXGW)
                    nc.gpsimd.dma_gather(
                        xgw[:, : w * H].rearrange("p (b e) -> p b e", e=H),
                        xsrc_ap,
                        srcW[:, g0 * (P // 16) : (g0 + w) * (P // 16)],
                        w * P, w * P, H,
                    )
                    gtiles[g0] = xgw

                def xg_slice(col, h):
                    g0 = (col // G) * G
                    issue_gather(g0)
                    issue_gather(g0 + G)
                    issue_gather(g0 + 2 * G)
                    k = col - g0
                    return gtiles[g0][:, k * H + h * P : k * H + (h + 1) * P]

                # S matrices, prefetched one chunk ahead
                stiles = {}

                def issue_S(col):
                    if col in stiles or col >= TC:
                        return
                    S = wp.tile([P, TILE2], gdt, name="S", tag="S", bufs=6)
                    nc.vector.tensor_scalar(
                        out=S[:], in0=iota_sb[:],
                        scalar1=dstlocT[:, col : col + 1],
                        scalar2=normT[:, col : col + 1],
                        op0=mybir.AluOpType.is_equal, op1=mybir.AluOpType.mult,
                    )
                    stiles[col] = S

                for t2 in range(T2):
                    rels = [r for r in range(R) if nch[t2 * R + r] > 0]
                    subs = [st for st in (2 * t2, 2 * t2 + 1) if st < TILES]
                    # self-loop: transpose own rows -> xT[h] = [fi, dst 256]
                    xT = {}
                    xown_sbs = []
                    for si_, st in enumerate(subs):
                        xown_sb = wp.tile([P, H], f32, name="xown", tag="xown", bufs=2)
                        nc.sync.dma_start(xown_sb[:], xown_ap[st * P : (st + 1) * P, :])
                        xown_sbs.append(xown_sb)
                    for h in range(2):
                        tp_ps = pp.tile([P, TILE2], f32, name="tp", tag="tpx", bufs=2)
                        for si_, st in enumerate(subs):
                            nc.tensor.transpose(
                                tp_ps[:, si_ * P : (si_ + 1) * P],
                                xown_sbs[si_][:, h * P : (h + 1) * P], ident[:],
                            )
                        xT_sb = wp.tile([P, TILE2], f32r, name=f"xT{h}", tag=f"xT{h}", bufs=2)
                        if h == 0:
                            nc.vector.tensor_copy(xT_sb[:], tp_ps[:])
                        else:
                            nc.scalar.copy(xT_sb[:], tp_ps[:])
                        xT[h] = xT_sb
                    # msgT accumulators [feat_out half, dst 256]
                    msgT_ps = {}
                    for ho in range(2):
                        msgT_ps[ho] = pp.tile(
                            [P, TILE2], f32, name=f"m{ho}", tag=f"m{ho}",
                            bufs=1,
                        )
                        for h in range(2):
                            nc.tensor.matmul(
                                msgT_ps[ho][:],
                                lhsT=loopw_ap(l, h)[:, ho * P : (ho + 1) * P],
                                rhs=xT[h][:],
                                start=(h == 0), stop=(h == 1 and not rels),
                            )

                    # rel-apply runs one cell late so the PSUM->SBUF copy of
                    # cell c overlaps cell c+1's chunk matmuls on the in-order
                    # PE queue instead of stalling it
                    pending = None

                    def flush_pending(stop):
                        nonlocal pending
                        if pending is None:
                            return
                        r_, sbs = pending
                        for h in range(2):
                            nc.tensor.matmul(
                                msgT_ps[h][:],
                                lhsT=wblk_ap(l, r_, h),
                                rhs=sbs[h][:],
                                start=False, stop=stop,
                            )
                        pending = None

                    for ri, r in enumerate(rels):
                        cell = t2 * R + r
                        cs = int(chunk_start[cell])
                        n = int(nch[cell])
                        aggT_ps = [
                            pp.tile([P, TILE2], f32, name=f"agg{h}", tag=f"agg{h}", bufs=2)
                            for h in range(2)
                        ]
                        for ci in range(n):
                            col = cs + ci
                            issue_S(col)
                            issue_S(col + 1)
                            issue_S(col + 2)
                            issue_S(col + 3)
                            xg0 = xg_slice(col, 0)
                            xg1 = xg_slice(col, 1)
                            S = stiles.pop(col)
                            nc.tensor.matmul(
                                aggT_ps[0][:], lhsT=xg0, rhs=S[:],
                                start=(ci == 0), stop=(ci == n - 1),
                            )
                            nc.tensor.matmul(
                                aggT_ps[1][:], lhsT=xg1, rhs=S[:],
                                start=(ci == 0), stop=(ci == n - 1),
                            )
                        sbs = []
                        for h in range(2):
                            aggT_sb = wp.tile(
                                [P, TILE2], f32r, name=f"aggsb{h}", tag=f"aggsb{h}", bufs=2
                            )
                            if (2 * cell + h) % 8 < 1:
                                nc.vector.tensor_copy(aggT_sb[:], aggT_ps[h][:])
                            else:
                                nc.scalar.copy(aggT_sb[:], aggT_ps[h][:])
                            sbs.append(aggT_sb)
                        flush_pending(stop=False)
                        pending = (r, sbs)
                    flush_pending(stop=True)
                    # transpose msgT back, add bias, store
                    out_sbs = [
                        wp.tile([P, H], f32, name=f"outsb{si_}", tag=f"outsb{si_}", bufs=2)
                        for si_ in range(len(subs))
                    ]
                    for ho in range(2):
                        msgT_sb = wp.tile(
                            [P, TILE2], f32, name=f"msgsb{ho}", tag=f"msgsb{ho}", bufs=2
                        )
                        if ho == 0:
                            nc.vector.tensor_copy(msgT_sb[:], msgT_ps[ho][:])
                        else:
                            nc.scalar.copy(msgT_sb[:], msgT_ps[ho][:])
                        for si_, st in enumerate(subs):
                            tp2_ps = pp.tile([P, P], f32, name="tp2", tag="tpx", bufs=2)
                            nc.tensor.transpose(
                                tp2_ps[:], msgT_sb[:, si_ * P : (si_ + 1) * P], ident[:]
                            )
                            nc.vector.tensor_tensor(
                                out=out_sbs[si_][:, ho * P : (ho + 1) * P],
                                in0=tp2_ps[:],
                                in1=biasbc_sb[:, l * H + ho * P : l * H + (ho + 1) * P],
                                op=mybir.AluOpType.add,
                            )
                    for si_, st in enumerate(subs):
                        out_sb = out_sbs[si_]
                        if out_pad_ap is not None:
                            nc.sync.dma_start(
                                out_pad_ap[st * P : (st + 1) * P, :], out_sb[:]
                            )
                        rows = min(P, NOWN - st * P)
                        pi = st // tpp
                        off = (st - pi * tpp) * P
                        if agdt[li] != f32:
                            out_b = wp.tile([P, H], bf16, name="outb", tag="outb", bufs=2)
                            nc.scalar.copy(out_b[:], out_sb[:])
                            nc.sync.dma_start(
                                agin[(li, pi)][off : off + rows, :], out_b[:rows, :]
                            )
                        else:
                            nc.sync.dma_start(
                                agin[(li, pi)][off : off + rows, :], out_sb[:rows, :]
                            )
                        # fire this piece's AllGather as soon as its tiles are
                        # stored so the collective overlaps the rest of the layer
                        if st == TILES - 1 or (st + 1) % tpp == 0:
                            emit_ag_piece(li, pi, h_full_out)

            layer(0, h0b_t.ap()[:], N, x0own_t.ap(), h1_own, 1, h1b_full)
            layer(1, h1b_full[:], N, h1_own, None, 2, h2_full)

            # ---- MLP head: all Q*P pairs of this core in one pass ----
            # gather drug/target rows (fp32) for all Q row-blocks at once
            xdr = wp.tile([P, Q * H], f32, name="xdr", tag="xdr", bufs=1)
            nc.gpsimd.dma_gather(
                xdr[:].rearrange("p (b e) -> p b e", e=H), h2_full[:],
                drugsW[:], Q * P, Q * P, H,
            )
            xtg = wp.tile([P, Q * H], f32, name="xtg", tag="xtg", bufs=1)
            nc.gpsimd.dma_gather(
                xtg[:].rearrange("p (b e) -> p b e", e=H), h2_full[:],
                targetsW[:], Q * P, Q * P, H,
            )
            NP = Q * P  # pairs per core
            # xcatT[k] = [feat chunk k of concat(drug,target), all pairs]
            xcatT = []
            for k in range(KC):
                src_sb = xdr if k < KC // 2 else xtg
                kk = k % (KC // 2)
                xcT = wp.tile([P, NP], f32r, name=f"xcT{k}", tag=f"xcT{k}", bufs=1)
                for q in range(Q):
                    tp2_ps = pp.tile([P, P], f32, name="tp2h", tag="tpx", bufs=2)
                    nc.tensor.transpose(
                        tp2_ps[:], src_sb[:, q * H + kk * P : q * H + (kk + 1) * P],
                        ident[:],
                    )
                    if q % 2 == 0:
                        nc.vector.tensor_copy(xcT[:, q * P : (q + 1) * P], tp2_ps[:])
                    else:
                        nc.scalar.copy(xcT[:, q * P : (q + 1) * P], tp2_ps[:])
                xcatT.append(xcT)
            z_ps = pp.tile([1, NP], f32, name="z", tag="m1", bufs=1)
            for m in range(MC):
                yT_ps = pp.tile([P, NP], f32, name="yT", tag="m0", bufs=1)
                for k in range(KC):
                    nc.tensor.matmul(
                        yT_ps[:],
                        lhsT=fc1_sb[:, (k * MC + m) * P : (k * MC + m + 1) * P],
                        rhs=xcatT[k][:],
                        start=(k == 0), stop=(k == KC - 1),
                    )
                yTr = wp.tile([P, NP], f32r, name="yTr", tag="yTr", bufs=2)
                nc.scalar.activation(
                    yTr[:], yT_ps[:], mybir.ActivationFunctionType.Relu,
                    bias=fc1b_sb[:, m : m + 1], scale=1.0,
                )
                nc.tensor.matmul(
                    z_ps[:], lhsT=fc2_sb[:, m : m + 1],
                    rhs=yTr[:],
                    start=(m == 0), stop=(m == MC - 1),
                )
            zs = wp.tile([1, NP], f32, name="zs", tag="zs", bufs=1)
            nc.scalar.activation(
                zs[:], z_ps[:], mybir.ActivationFunctionType.Sigmoid,
                bias=meta["fc2b"], scale=1.0,
            )
            nc.sync.dma_start(out_t.ap()[:, :], zs[:])
    return nc


_NC_CACHE = []


def kernel(**inputs):
    from concourse import bass_utils

    meta, in_maps = _preprocess(inputs)
    key = (meta["N"], meta["H"], meta["R"], meta["TC"], meta["Q"],
           tuple(int(x) for x in meta["nch"]))
    if _NC_CACHE and _NC_CACHE[0][0] == key:
        nc = _NC_CACHE[0][1]
    else:
        nc = _build(meta)
        nc.compile()
        _NC_CACHE[:] = [(key, nc)]
    res = bass_utils.run_bass_kernel_spmd(nc, in_maps, core_ids=list(range(NCORES)))
    out = np.concatenate([res.results[c]["out"] for c in range(NCORES)], axis=0)
    return out.astype(np.float32)


# revision 21
# speedup vs baseline: 1.3492x; 1.0124x over previous
"""Trainium2 Bass kernel for the DTI R-GCN (bdd) model, 8 NeuronCores.

Strategy (SPMD, one program, per-core data):
  - dst-shard the graph: core c owns nodes [c*2500, (c+1)*2500); host routes
    each edge to its dst owner and buckets it into (dst-tile, relation) cells,
    tiles of 256 dst nodes (TILE2).
  - edge chunks of 128 are gathered in batches of G=8 chunks per
    indirect-DMA call (amortizes the ~1us SWDGE fixed cost) from a bf16 copy
    of the node features; a scatter matrix S[e, d] = norm_e * (iota ==
    dstloc_e) is built per chunk (fp32), and aggT[feat, dst] accumulates in
    PSUM via matmul(lhsT=xg_half[bf16], rhs=S[f32r]) at 1 cycle/row.
  - per dst-tile, msgT[feat_out, dst 256] accumulates in PSUM: the self-loop
    term matmul(lhsT=loopw[f32r], rhs=xT[f32r]) plus one matmul per relation
    matmul(lhsT=Wblk_r[f32r], rhs=aggT_sb[f32r]) -- all free-dim 256 so f32r
    runs at full rate with full fp32 data.  msgT is PE-transposed back,
    bias-added, and stored (fp32 own rows for the next layer's self-loop +
    bf16 rows for the next layer's gathers).
  - layer outputs are AllGather'd piecewise (layer1 in bf16 for the gathers,
    layer2 in fp32 for the MLP head).
  - MLP head is data-parallel over pairs: all 512 per-core pairs in one pass
    with free-dim-512 f32r matmuls.

The program is built fresh per invocation from the actual inputs (host does
all index preprocessing; trip counts are data-dependent but identical across
cores by padding cells to the max chunk count over cores).
"""
import sys

sys.path.insert(0, "/opt/trn_rl_repo")
import numpy as np
import ml_dtypes

P = 128
TILE2 = 2 * P  # dst nodes per aggregation cell


def _round_f32r(x):
    """Round fp32 -> FP32R (1s/8e/11m, low 12 mantissa bits zero, RNE) so
    DRAM uploads declared float32r hold already-rounded values."""
    u = np.ascontiguousarray(x, np.float32).view(np.uint32)
    u = (u + 0x7FF + ((u >> 12) & 1)) & np.uint32(0xFFFFF000)
    return u.view(np.float32)

G = 8  # chunks gathered per indirect-DMA call
# aggregation matmul dtype: bf16 halves gather DMA + S-build cost but rounds
# the gathered features and edge norms; f32(r) is exact (walrus forbids
# mixing, so gathers/S/matmul all follow this flag together)
AGG_BF16 = True
NCORES = 8
OOB = np.int32(2**28)


def _preprocess(inputs, ncores=NCORES):
    node_ids = np.asarray(inputs["node_ids"])
    src = np.asarray(inputs["src"])
    dst = np.asarray(inputs["dst"])
    etype = np.asarray(inputs["etype"])
    norm = np.asarray(inputs["norm"]).reshape(-1)
    emb = np.asarray(inputs["emb"], dtype=np.float32)
    drugs = np.asarray(inputs["drugs_index"])
    targets = np.asarray(inputs["targets_index"])

    N = node_ids.shape[0]
    H = emb.shape[1]
    R = int(inputs["w1"].shape[0])
    PAIRS = drugs.shape[0]
    assert N % ncores == 0 and PAIRS % ncores == 0
    NOWN = N // ncores
    TILES = -(-NOWN // P)
    NCELL = (-(-NOWN // TILE2)) * R
    PPC = PAIRS // ncores
    assert PPC % P == 0
    Q = PPC // P

    T2 = -(-NOWN // TILE2)
    owner = dst // NOWN
    d_local = dst - owner * NOWN
    t_of_e = d_local // TILE2
    dstloc_of_e = (d_local % TILE2).astype(np.float32)
    cell_of_e = t_of_e * R + etype

    counts = np.zeros((ncores, NCELL), np.int64)
    for c in range(ncores):
        counts[c] = np.bincount(cell_of_e[owner == c], minlength=NCELL)
    nch = -(-counts.max(axis=0) // P)  # chunks per cell (0 if empty everywhere)
    chunk_start = np.zeros(NCELL, np.int64)
    chunk_start[1:] = np.cumsum(nch)[:-1]
    TC = int(nch.sum())

    srcT = np.zeros((ncores, P, TC), np.int32)  # pads point at row 0 (S zeroes them)
    dstlocT = np.zeros((ncores, P, TC), np.float32)
    normT = np.zeros((ncores, P, TC), np.float32)
    for c in range(ncores):
        m = owner == c
        eidx = np.where(m)[0]
        cell = cell_of_e[eidx]
        order = np.argsort(cell, kind="stable")
        eidx = eidx[order]
        cell = cell[order]
        cstart = np.zeros(NCELL, np.int64)
        cstart[1:] = np.cumsum(counts[c])[:-1]
        rank = np.arange(len(eidx)) - cstart[cell]
        col = chunk_start[cell] + rank // P
        part = rank % P
        srcT[c, part, col] = src[eidx]
        dstlocT[c, part, col] = dstloc_of_e[eidx]
        normT[c, part, col] = norm[eidx]

    # host-side embedding lookup: pure data movement, shrinks per-core upload
    # from the full table to the active [N, H] slab
    h0 = emb[node_ids]  # [N, H]
    h0b = h0.astype(ml_dtypes.bfloat16) if AGG_BF16 else _round_f32r(h0)  # gather table
    x0own = np.zeros((ncores, TILES * P, H), np.float32)
    for c in range(ncores):
        x0own[c, :NOWN] = h0[c * NOWN : (c + 1) * NOWN]

    def _wrap16(flat):
        # dma_gather index layout: flat j -> partition j%16, column j//16,
        # replicated across all 128 partitions
        assert len(flat) % 16 == 0
        w = np.asarray(flat, np.int16).reshape(-1, 16).T.copy()
        return np.tile(w, (8, 1))

    srcW = np.stack([_wrap16(srcT[c].T.ravel()) for c in range(ncores)])
    drugsW = np.stack([_wrap16(drugs[c * PPC : (c + 1) * PPC]) for c in range(ncores)])
    targetsW = np.stack([_wrap16(targets[c * PPC : (c + 1) * PPC]) for c in range(ncores)])

    # relation block weights as lhsT [if_local, of_local] per (layer, rel, half)
    B = int(inputs["w1"].shape[1])
    si = H // B
    hb = (P // si)  # blocks per half
    wblk = np.zeros((2, R, 2, P, P), np.float32)
    for l, W in enumerate([inputs["w1"], inputs["w2"]]):
        W = np.asarray(W, np.float32)
        for r in range(R):
            for h in range(2):
                for bb in range(hb):
                    b = hb * h + bb
                    wblk[l, r, h, bb * si : (bb + 1) * si, bb * si : (bb + 1) * si] = W[r, b]
    wblk_in = _round_f32r(wblk.transpose(3, 0, 1, 2, 4).reshape(P, 2 * R * 2 * P))

    loopw = np.stack(
        [np.asarray(inputs["loop_w1"], np.float32), np.asarray(inputs["loop_w2"], np.float32)]
    )  # [2, H, H]
    loopw_in = _round_f32r(loopw.reshape(2, 2, P, H).transpose(2, 0, 1, 3).reshape(P, 2 * 2 * H))

    bias_in = np.concatenate(
        [
            np.tile(np.asarray(inputs["b1"], np.float32)[None, :], (P, 1)),
            np.tile(np.asarray(inputs["b2"], np.float32)[None, :], (P, 1)),
        ],
        axis=1,
    )  # [P, 2H]

    d2 = 2 * H
    KC = d2 // P  # fc1 contraction chunks
    MC = d2 // P  # fc1 output chunks
    fc1_in = _round_f32r(
        np.asarray(inputs["fc1_W"], np.float32)
        .reshape(KC, P, MC, P)
        .transpose(1, 0, 2, 3)
        .reshape(P, KC * MC * P)
    )
    fc1b_in = np.asarray(inputs["fc1_b"], np.float32).reshape(MC, P).T.copy()
    fc2_in = _round_f32r(np.asarray(inputs["fc2_W"], np.float32).reshape(MC, P).T)
    fc2b = float(np.asarray(inputs["fc2_b"]).reshape(-1)[0])

    iota = np.tile(np.arange(TILE2, dtype=ml_dtypes.bfloat16), (P, 1))

    meta = dict(
        N=N, H=H, R=R, NOWN=NOWN, TILES=TILES, T2=T2, NCELL=NCELL, TC=TC, Q=Q,
        KC=KC, MC=MC, nch=nch, chunk_start=chunk_start, fc2b=fc2b,
    )
    shared = dict(
        h0b=h0b, iota=iota, wblk=wblk_in, loopw=loopw_in, biasbc=bias_in,
        fc1=fc1_in, fc1b=fc1b_in, fc2=fc2_in,
    )
    in_maps = []
    for c in range(ncores):
        m = dict(shared)
        m.update(
            srcW=srcW[c], dstlocT=dstlocT[c], normT=normT[c],
            x0own=x0own[c], drugsW=drugsW[c], targetsW=targetsW[c],
        )
        in_maps.append(m)
    return meta, in_maps


def _build(meta, ncores=NCORES, single=False):
    from concourse import bass, mybir, tile, bacc
    from concourse.masks import make_identity

    N, H, R = meta["N"], meta["H"], meta["R"]
    NOWN, TILES, TC, Q = meta["NOWN"], meta["TILES"], meta["TC"], meta["Q"]
    T2 = meta["T2"]
    KC, MC = meta["KC"], meta["MC"]
    nch, chunk_start = meta["nch"], meta["chunk_start"]
    f32 = mybir.dt.float32
    f32r = mybir.dt.float32r
    bf16 = mybir.dt.bfloat16
    i32 = mybir.dt.int32
    gdt = bf16 if AGG_BF16 else f32r  # gather-table / xg / S dtype

    nc = bacc.Bacc(
        "TRN2", target_bir_lowering=False, debug=False,
        num_devices=(1 if single else ncores),
        dynamic_dma_scratch_size=32768,
    )

    h0b_t = nc.dram_tensor("h0b", [N, H], gdt, kind="ExternalInput")
    i16 = mybir.dt.int16
    WTC = TC * P // 16
    srcW_t = nc.dram_tensor("srcW", [P, WTC], i16, kind="ExternalInput")
    dstlocT_t = nc.dram_tensor("dstlocT", [P, TC], f32, kind="ExternalInput")
    normT_t = nc.dram_tensor("normT", [P, TC], f32, kind="ExternalInput")
    x0own_t = nc.dram_tensor("x0own", [TILES * P, H], f32, kind="ExternalInput")
    WQ = Q * P // 16
    drugsW_t = nc.dram_tensor("drugsW", [P, WQ], i16, kind="ExternalInput")
    targetsW_t = nc.dram_tensor("targetsW", [P, WQ], i16, kind="ExternalInput")
    iota_t = nc.dram_tensor("iota", [P, TILE2], bf16, kind="ExternalInput")
    wblk_t = nc.dram_tensor("wblk", [P, 2 * R * 2 * P], f32r, kind="ExternalInput")
    loopw_t = nc.dram_tensor("loopw", [P, 2 * 2 * H], f32r, kind="ExternalInput")
    biasbc_t = nc.dram_tensor("biasbc", [P, 2 * H], f32, kind="ExternalInput")
    fc1_t = nc.dram_tensor("fc1", [P, KC * MC * P], f32r, kind="ExternalInput")
    fc1b_t = nc.dram_tensor("fc1b", [P, MC], f32, kind="ExternalInput")
    fc2_t = nc.dram_tensor("fc2", [P, MC], f32r, kind="ExternalInput")
    out_t = nc.dram_tensor("out", [Q * P, 1], f32, kind="ExternalOutput")

    with tile.TileContext(nc) as tc:
        with (
            tc.tile_pool(name="const", bufs=1) as cp,
            tc.tile_pool(name="work", bufs=2) as wp,
            tc.tile_pool(name="ps", bufs=1, space="PSUM") as pp,
        ):
            # ---- DRAM internals; AllGather split into piece collectives so
            # each piece fires as soon as its tiles are stored (overlaps the
            # rest of the layer), then one strided DMA folds it into the
            # node-indexed full table.
            tpp = max(1, -(-TILES // 4))  # tiles per AG piece
            pieces = []  # (row0, nrows)
            for p0 in range(0, TILES, tpp):
                row0 = p0 * P
                nrows = min(NOWN, (p0 + tpp) * P) - row0
                if nrows > 0:
                    pieces.append((row0, nrows))
            h1_own = nc.dram_tensor("h1_own", [TILES * P, H], f32, kind="Internal").ap()
            h1b_full = nc.dram_tensor("h1b_full", [N, H], gdt, kind="Internal").ap()
            h2_full = nc.dram_tensor("h2_full", [N, H], f32, kind="Internal").ap()
            agdt = {1: gdt, 2: f32}
            agin = {}
            agout = {}
            for li in (1, 2):
                for pi, (row0, nrows) in enumerate(pieces):
                    agin[(li, pi)] = nc.dram_tensor(
                        f"h{li}_agin{pi}", [nrows, H], agdt[li], kind="Internal"
                    ).ap()
                    agout[(li, pi)] = nc.dram_tensor(
                        f"h{li}_agout{pi}", [ncores * nrows, H], agdt[li],
                        kind="Internal", addr_space="Shared",
                    ).ap()

            # ---- resident constants ----
            srcW = cp.tile([P, WTC], i16, name="srcW")
            nc.sync.dma_start(srcW[:], srcW_t.ap()[:])
            dstlocT = cp.tile([P, TC], f32, name="dstlocT")
            nc.sync.dma_start(dstlocT[:], dstlocT_t.ap()[:])
            normT = cp.tile([P, TC], f32, name="normT")
            nc.sync.dma_start(normT[:], normT_t.ap()[:])
            drugsW = cp.tile([P, WQ], i16, name="drugsW")
            nc.sync.dma_start(drugsW[:], drugsW_t.ap()[:])
            targetsW = cp.tile([P, WQ], i16, name="targetsW")
            nc.sync.dma_start(targetsW[:], targetsW_t.ap()[:])
            iota_sb = cp.tile([P, TILE2], bf16, name="iota_sb")
            nc.sync.dma_start(iota_sb[:], iota_t.ap()[:])
            wblk_sb = cp.tile([P, 2 * R * 2 * P], f32r, name="wblk_sb")
            nc.sync.dma_start(wblk_sb[:], wblk_t.ap()[:])
            loopw_sb = cp.tile([P, 2 * 2 * H], f32r, name="loopw_sb")
            nc.sync.dma_start(loopw_sb[:], loopw_t.ap()[:])
            biasbc_sb = cp.tile([P, 2 * H], f32, name="biasbc_sb")
            nc.sync.dma_start(biasbc_sb[:], biasbc_t.ap()[:])
            fc1_sb = cp.tile([P, KC * MC * P], f32r, name="fc1_sb")
            nc.sync.dma_start(fc1_sb[:], fc1_t.ap()[:])
            fc1b_sb = cp.tile([P, MC], f32, name="fc1b_sb")
            nc.sync.dma_start(fc1b_sb[:], fc1b_t.ap()[:])
            fc2_sb = cp.tile([P, MC], f32r, name="fc2_sb")
            nc.sync.dma_start(fc2_sb[:], fc2_t.ap()[:])
            ident = cp.tile([P, P], f32, name="ident")
            make_identity(nc, ident[:])

            def wblk_ap(l, r, h):
                o = ((l * R + r) * 2 + h) * P
                return wblk_sb[:, o : o + P]

            def loopw_ap(l, h):
                o = (l * 2 + h) * H
                return loopw_sb[:, o : o + H]

            XGW = G * H  # wide gather tile columns
            NXGW = 8

            def emit_ag_piece(li, pi, h_full):
                row0, nrows = pieces[pi]
                if single:
                    nc.sync.dma_start(
                        h_full[row0 : row0 + nrows, :], agin[(li, pi)][:]
                    )
                    return
                nc.gpsimd.collective_compute(
                    "AllGather", mybir.AluOpType.bypass,
                    replica_groups=[list(range(ncores))],
                    ins=[agin[(li, pi)]], outs=[agout[(li, pi)]],
                )
                src_ap = agout[(li, pi)].rearrange("(c n) h -> c n h", c=ncores)
                dst_ap = h_full.rearrange("(c n) h -> c n h", c=ncores)[
                    :, row0 : row0 + nrows, :
                ]
                nc.sync.dma_start(dst_ap, src_ap)

            def layer(l, xsrc_ap, xsrc_rows, xown_ap, out_pad_ap, li, h_full_out):
                # gather groups of G chunk columns, prefetched one group ahead
                gtiles = {}

                def issue_gather(g0):
                    if g0 in gtiles or g0 >= TC:
                        return
                    w = min(G, TC - g0)
                    xgw = wp.tile([P, XGW], gdt, name="xgw", tag="xgw", bufs=NXGW)
                    nc.gpsimd.dma_gather(
                        xgw[:, : w * H].rearrange("p (b e) -> p b e", e=H),
                        xsrc_ap,
                        srcW[:, g0 * (P // 16) : (g0 + w) * (P // 16)],
                        w * P, w * P, H,
                    )
                    gtiles[g0] = xgw

                def xg_slice(col, h):
                    g0 = (col // G) * G
                    issue_gather(g0)
                    issue_gather(g0 + G)
                    issue_gather(g0 + 2 * G)
                    issue_gather(g0 + 3 * G)
                    k = col - g0
                    return gtiles[g0][:, k * H + h * P : k * H + (h + 1) * P]

                # S matrices, prefetched one chunk ahead
                stiles = {}

                def issue_S(col):
                    if col in stiles or col >= TC:
                        return
                    S = wp.tile([P, TILE2], gdt, name="S", tag="S", bufs=6)
                    nc.vector.tensor_scalar(
                        out=S[:], in0=iota_sb[:],
                        scalar1=dstlocT[:, col : col + 1],
                        scalar2=normT[:, col : col + 1],
                        op0=mybir.AluOpType.is_equal, op1=mybir.AluOpType.mult,
                    )
                    stiles[col] = S

                for t2 in range(T2):
                    rels = [r for r in range(R) if nch[t2 * R + r] > 0]
                    subs = [st for st in (2 * t2, 2 * t2 + 1) if st < TILES]
                    # self-loop: transpose own rows -> xT[h] = [fi, dst 256]
                    xT = {}
                    xown_sbs = []
                    for si_, st in enumerate(subs):
                        xown_sb = wp.tile([P, H], f32, name="xown", tag="xown", bufs=2)
                        nc.sync.dma_start(xown_sb[:], xown_ap[st * P : (st + 1) * P, :])
                        xown_sbs.append(xown_sb)
                    for h in range(2):
                        tp_ps = pp.tile([P, TILE2], f32, name="tp", tag="tpx", bufs=2)
                        for si_, st in enumerate(subs):
                            nc.tensor.transpose(
                                tp_ps[:, si_ * P : (si_ + 1) * P],
                                xown_sbs[si_][:, h * P : (h + 1) * P], ident[:],
                            )
                        xT_sb = wp.tile([P, TILE2], f32r, name=f"xT{h}", tag=f"xT{h}", bufs=2)
                        if h == 0:
                            nc.vector.tensor_copy(xT_sb[:], tp_ps[:])
                        else:
                            nc.scalar.copy(xT_sb[:], tp_ps[:])
                        xT[h] = xT_sb
                    # msgT accumulators [feat_out half, dst 256]; the
                    # self-loop matmuls are emitted at tile end (emit_loop) so
                    # they cover the last cell's PSUM->SBUF copy latency
                    msgT_ps = {}
                    for ho in range(2):
                        msgT_ps[ho] = pp.tile(
                            [P, TILE2], f32, name=f"m{ho}", tag=f"m{ho}",
                            bufs=1,
                        )

                    def emit_loop(start):
                        for ho in range(2):
                            for h in range(2):
                                nc.tensor.matmul(
                                    msgT_ps[ho][:],
                                    lhsT=loopw_ap(l, h)[:, ho * P : (ho + 1) * P],
                                    rhs=xT[h][:],
                                    start=(start and h == 0),
                                    stop=(not rels and h == 1),
                                )

                    # rel-apply runs one cell late so the PSUM->SBUF copy of
                    # cell c overlaps cell c+1's chunk matmuls on the in-order
                    # PE queue instead of stalling it
                    pending = None

                    first_flush = [True]

                    def flush_pending(stop):
                        nonlocal pending
                        if pending is None:
                            return
                        r_, sbs = pending
                        for h in range(2):
                            nc.tensor.matmul(
                                msgT_ps[h][:],
                                lhsT=wblk_ap(l, r_, h),
                                rhs=sbs[h][:],
                                start=first_flush[0], stop=stop,
                            )
                        first_flush[0] = False
                        pending = None

                    for ri, r in enumerate(rels):
                        cell = t2 * R + r
                        cs = int(chunk_start[cell])
                        n = int(nch[cell])
                        aggT_ps = [
                            pp.tile([P, TILE2], f32, name=f"agg{h}", tag=f"agg{h}", bufs=2)
                            for h in range(2)
                        ]
                        for ci in range(n):
                            col = cs + ci
                            issue_S(col)
                            issue_S(col + 1)
                            issue_S(col + 2)
                            issue_S(col + 3)
                            xg0 = xg_slice(col, 0)
                            xg1 = xg_slice(col, 1)
                            S = stiles.pop(col)
                            nc.tensor.matmul(
                                aggT_ps[0][:], lhsT=xg0, rhs=S[:],
                                start=(ci == 0), stop=(ci == n - 1),
                            )
                            nc.tensor.matmul(
                                aggT_ps[1][:], lhsT=xg1, rhs=S[:],
                                start=(ci == 0), stop=(ci == n - 1),
                            )
                        sbs = []
                        for h in range(2):
                            aggT_sb = wp.tile(
                                [P, TILE2], f32r, name=f"aggsb{h}", tag=f"aggsb{h}", bufs=2
                            )
                            if (2 * cell + h) % 8 < 1:
                                nc.vector.tensor_copy(aggT_sb[:], aggT_ps[h][:])
                            else:
                                nc.scalar.copy(aggT_sb[:], aggT_ps[h][:])
                            sbs.append(aggT_sb)
                        flush_pending(stop=False)
                        pending = (r, sbs)
                    emit_loop(start=not rels or first_flush[0])
                    flush_pending(stop=True)
                    # transpose msgT back, add bias, store
                    out_sbs = [
                        wp.tile([P, H], f32, name=f"outsb{si_}", tag=f"outsb{si_}", bufs=2)
                        for si_ in range(len(subs))
                    ]
                    for ho in range(2):
                        msgT_sb = wp.tile(
                            [P, TILE2], f32, name=f"msgsb{ho}", tag=f"msgsb{ho}", bufs=2
                        )
                        if ho == 0:
                            nc.vector.tensor_copy(msgT_sb[:], msgT_ps[ho][:])
                        else:
                            nc.scalar.copy(msgT_sb[:], msgT_ps[ho][:])
                        for si_, st in enumerate(subs):
                            tp2_ps = pp.tile([P, P], f32, name="tp2", tag="tpx", bufs=2)
                            nc.tensor.transpose(
                                tp2_ps[:], msgT_sb[:, si_ * P : (si_ + 1) * P], ident[:]
                            )
                            nc.vector.tensor_tensor(
                                out=out_sbs[si_][:, ho * P : (ho + 1) * P],
                                in0=tp2_ps[:],
                                in1=biasbc_sb[:, l * H + ho * P : l * H + (ho + 1) * P],
                                op=mybir.AluOpType.add,
                            )
                    for si_, st in enumerate(subs):
                        out_sb = out_sbs[si_]
                        if out_pad_ap is not None:
                            nc.sync.dma_start(
                                out_pad_ap[st * P : (st + 1) * P, :], out_sb[:]
                            )
                        rows = min(P, NOWN - st * P)
                        pi = st // tpp
                        off = (st - pi * tpp) * P
                        if agdt[li] != f32:
                            out_b = wp.tile([P, H], bf16, name="outb", tag="outb", bufs=2)
                            nc.scalar.copy(out_b[:], out_sb[:])
                            nc.sync.dma_start(
                                agin[(li, pi)][off : off + rows, :], out_b[:rows, :]
                            )
                        else:
                            nc.sync.dma_start(
                                agin[(li, pi)][off : off + rows, :], out_sb[:rows, :]
                            )
                        # fire this piece's AllGather as soon as its tiles are
                        # stored so the collective overlaps the rest of the layer
                        if st == TILES - 1 or (st + 1) % tpp == 0:
                            emit_ag_piece(li, pi, h_full_out)

            layer(0, h0b_t.ap()[:], N, x0own_t.ap(), h1_own, 1, h1b_full)
            layer(1, h1b_full[:], N, h1_own, None, 2, h2_full)

            # ---- MLP head: all Q*P pairs of this core in one pass ----
            # gather drug/target rows (fp32) for all Q row-blocks at once
            xdr = wp.tile([P, Q * H], f32, name="xdr", tag="xdr", bufs=1)
            nc.gpsimd.dma_gather(
                xdr[:].rearrange("p (b e) -> p b e", e=H), h2_full[:],
                drugsW[:], Q * P, Q * P, H,
            )
            xtg = wp.tile([P, Q * H], f32, name="xtg", tag="xtg", bufs=1)
            nc.gpsimd.dma_gather(
                xtg[:].rearrange("p (b e) -> p b e", e=H), h2_full[:],
                targetsW[:], Q * P, Q * P, H,
            )
            NP = Q * P  # pairs per core
            # xcatT[k] = [feat chunk k of concat(drug,target), all pairs]
            xcatT = []
            for k in range(KC):
                src_sb = xdr if k < KC // 2 else xtg
                kk = k % (KC // 2)
                xcT = wp.tile([P, NP], f32r, name=f"xcT{k}", tag=f"xcT{k}", bufs=1)
                for q in range(Q):
                    tp2_ps = pp.tile([P, P], f32, name="tp2h", tag="tpx", bufs=2)
                    nc.tensor.transpose(
                        tp2_ps[:], src_sb[:, q * H + kk * P : q * H + (kk + 1) * P],
                        ident[:],
                    )
                    if q % 2 == 0:
                        nc.vector.tensor_copy(xcT[:, q * P : (q + 1) * P], tp2_ps[:])
                    else:
                        nc.scalar.copy(xcT[:, q * P : (q + 1) * P], tp2_ps[:])
                xcatT.append(xcT)
            z_ps = pp.tile([1, NP], f32, name="z", tag="m1", bufs=1)
            for m in range(MC):
                yT_ps = pp.tile([P, NP], f32, name="yT", tag="m0", bufs=1)
                for k in range(KC):
                    nc.tensor.matmul(
                        yT_ps[:],
                        lhsT=fc1_sb[:, (k * MC + m) * P : (k * MC + m + 1) * P],
                        rhs=xcatT[k][:],
                        start=(k == 0), stop=(k == KC - 1),
                    )
                yTr = wp.tile([P, NP], f32r, name="yTr", tag="yTr", bufs=2)
                nc.scalar.activation(
                    yTr[:], yT_ps[:], mybir.ActivationFunctionType.Relu,
                    bias=fc1b_sb[:, m : m + 1], scale=1.0,
                )
                nc.tensor.matmul(
                    z_ps[:], lhsT=fc2_sb[:, m : m + 1],
                    rhs=yTr[:],
                    start=(m == 0), stop=(m == MC - 1),
                )
            zs = wp.tile([1, NP], f32, name="zs", tag="zs", bufs=1)
            nc.scalar.activation(
                zs[:], z_ps[:], mybir.ActivationFunctionType.Sigmoid,
                bias=meta["fc2b"], scale=1.0,
            )
            nc.sync.dma_start(out_t.ap()[:, :], zs[:])
    return nc


_NC_CACHE = []


def kernel(**inputs):
    from concourse import bass_utils

    meta, in_maps = _preprocess(inputs)
    key = (meta["N"], meta["H"], meta["R"], meta["TC"], meta["Q"],
           tuple(int(x) for x in meta["nch"]))
    if _NC_CACHE and _NC_CACHE[0][0] == key:
        nc = _NC_CACHE[0][1]
    else:
        nc = _build(meta)
        nc.compile()
        _NC_CACHE[:] = [(key, nc)]
    res = bass_utils.run_bass_kernel_spmd(nc, in_maps, core_ids=list(range(NCORES)))
    out = np.concatenate([res.results[c]["out"] for c in range(NCORES)], axis=0)
    return out.astype(np.float32)
